# revision 7
# baseline (speedup 1.0000x reference)
"""CubECC2d Trainium kernel.

Computes per-image Euler Characteristic Curves for a cubical complex
(V-construction, sublevel filtration) over x: [128, 3, 256, 256] f32.
out[b, c, s] = #V(<=th_s) - #Eh(<=th_s) - #Ev(<=th_s) + #S(<=th_s),
i.e. the cumsum of the binned Euler characteristic histogram.

8-core data parallelism (48 images/core).  Per image the device builds the
four cell arrays (vertex values, horizontal/vertical edge maxes, square
maxes) and, for each of 64 exact fp32 thresholds, counts signed cells in
two fused custom-DVE instructions (ECC_COUNT: (in0<th)-(in1<th), reduced
along the free dim with a chainable accumulator).  A PE ones-matmul
reduces the per-partition counts.  Thresholds are precomputed on the host
by bit-exact binary search so that (x < th'_s) exactly reproduces the
reference's ceil-based binning; no arithmetic touches x on device.
"""

import numpy as np

B, C, H, W = 128, 3, 256, 256
STEPS = 64
T_MIN, T_MAX = 0.02, 0.98
RES = (T_MAX - T_MIN) / (STEPS - 1)

N_CORES = 8
IMG_PER_CORE = (B * C) // N_CORES  # 48
PAD = 2.0  # sentinel > every threshold; padded slots never counted

_cache = {}


def _bin_of(v):
    """g(v) in 0..64 replicating reference binning: cell with filtration v is
    counted in cum[s] iff g(v) <= s."""
    v = np.float32(v)
    if v > np.float32(T_MAX):
        return 64
    u = (v - np.float32(T_MIN)) / np.float32(RES)
    t = int(np.ceil(u))
    return min(max(t, 0), STEPS - 1)


def _thetas_strict():
    """th'[s] = smallest float32 with g(v) > s, so (v < th'[s]) <=> g(v) <= s
    for all float32 v in [0, 1)."""
    one_bits = int(np.float32(1.0).view(np.uint32))
    out = np.empty(STEPS, np.float32)
    for s in range(STEPS):
        lo, hi = 0, one_bits - 1
        assert _bin_of(np.uint32(lo).view(np.float32)) <= s
        if _bin_of(np.uint32(hi).view(np.float32)) <= s:
            # everything below 1.0 counts; next float is 1.0 itself
            out[s] = np.float32(1.0)
            continue
        while hi - lo > 1:
            mid = (lo + hi) // 2
            if _bin_of(np.uint32(mid).view(np.float32)) <= s:
                lo = mid
            else:
                hi = mid
        out[s] = np.uint32(hi).view(np.float32)
    return out


def _build(n_img, h, w):
    """Per-core program.  Inputs: x [n_img, h, w] f32, padrow [1, w] f32.
    Output: out [n_img, 64] f32 (the finished ECC curves)."""
    from contextlib import ExitStack

    import concourse.tile as tile
    from concourse import bacc, mybir
    from ecc_op import ECC_COUNT

    assert h % 2 == 0 and w % 2 == 0
    hp = h // 2
    fw = 2 * w

    nc = bacc.Bacc("TRN2", target_bir_lowering=False, debug=False)
    x = nc.dram_tensor("x", [n_img, h, w], mybir.dt.float32, kind="ExternalInput").ap()
    padrow = nc.dram_tensor("padrow", [1, w], mybir.dt.float32, kind="ExternalInput").ap()
    out = nc.dram_tensor("out", [n_img, 64], mybir.dt.float32, kind="ExternalOutput").ap()

    f32 = mybir.dt.float32
    OP = mybir.AluOpType
    ths = _thetas_strict()

    with tile.TileContext(nc) as tc:
        with ExitStack() as ctx:
            const_pool = ctx.enter_context(tc.tile_pool(name="const", bufs=1))
            img_pool = ctx.enter_context(tc.tile_pool(name="img", bufs=3))
            arr_pool = ctx.enter_context(tc.tile_pool(name="arr", bufs=3))
            acc_pool = ctx.enter_context(tc.tile_pool(name="acc", bufs=3))
            scr_pool = ctx.enter_context(tc.tile_pool(name="scr", bufs=4))
            psum_pool = ctx.enter_context(
                tc.tile_pool(name="psum", bufs=3, space="PSUM")
            )

            ones = const_pool.tile([hp, 1], f32)
            nc.vector.memset(ones[:], 1.0)

            for img in range(n_img):
                X = img_pool.tile([hp, fw], f32, tag="X")
                nc.sync.dma_start(X[:, 0:w], x[img, 0:hp, :])
                nc.sync.dma_start(X[:, w : 2 * w], x[img, hp : 2 * hp, :])
                Xd = img_pool.tile([hp, fw], f32, tag="Xd")
                nc.sync.dma_start(Xd[:, 0:w], x[img, 1 : hp + 1, :])
                nc.sync.dma_start(Xd[0 : hp - 1, w : 2 * w], x[img, hp + 1 : 2 * hp, :])
                nc.sync.dma_start(Xd[hp - 1 : hp, w : 2 * w], padrow[:, :])

                X3 = X[:].rearrange("p (b j) -> p b j", b=2)
                Eh = arr_pool.tile([hp, fw], f32, tag="Eh")
                Eh3 = Eh[:].rearrange("p (b j) -> p b j", b=2)
                nc.vector.tensor_tensor(
                    Eh3[:, :, 0 : w - 1], X3[:, :, 0 : w - 1], X3[:, :, 1:w], op=OP.max
                )
                nc.vector.memset(Eh3[:, :, w - 1 : w], PAD)
                Ev = arr_pool.tile([hp, fw], f32, tag="Ev")
                nc.vector.tensor_tensor(Ev[:], X[:], Xd[:], op=OP.max)
                S = arr_pool.tile([hp, fw], f32, tag="S")
                Ev3 = Ev[:].rearrange("p (b j) -> p b j", b=2)
                S3 = S[:].rearrange("p (b j) -> p b j", b=2)
                nc.vector.tensor_tensor(
                    S3[:, :, 0 : w - 1], Ev3[:, :, 0 : w - 1], Ev3[:, :, 1:w], op=OP.max
                )
                nc.vector.memset(S3[:, :, w - 1 : w], PAD)

                acc = acc_pool.tile([hp, STEPS], f32, tag="acc")
                for s in range(STEPS):
                    th = float(ths[s])
                    scr = scr_pool.tile([hp, fw], f32, tag="scr")
                    nc.vector._custom_dve(
                        ECC_COUNT,
                        out=scr[:],
                        in0=X[:],
                        in1=Eh[:],
                        s0=th,
                        s1=0.0,
                        accum_out=acc[:, s : s + 1],
                    )
                    scr = scr_pool.tile([hp, fw], f32, tag="scr")
                    nc.vector._custom_dve(
                        ECC_COUNT,
                        out=scr[:],
                        in0=S[:],
                        in1=Ev[:],
                        s0=th,
                        s1=acc[:, s : s + 1],
                        accum_out=acc[:, s : s + 1],
                    )
                # partition-reduce: [1, 64] = ones.T @ acc
                ps = psum_pool.tile([1, STEPS], f32, tag="ps")
                nc.tensor.matmul(ps[:], ones[:], acc[:], start=True, stop=True)
                cur = acc_pool.tile([1, STEPS], f32, tag="cur")
                nc.scalar.copy(cur[:], ps[:])
                nc.sync.dma_start(out[img : img + 1, :], cur[:])

    nc.compile()
    return nc


def _get_nc(n_img, h, w):
    key = (n_img, h, w)
    if key not in _cache:
        _cache[key] = _build(n_img, h, w)
    return _cache[key]


def _in_maps(x):
    xr = x.reshape(N_CORES, IMG_PER_CORE, H, W)
    pr = np.full((1, W), PAD, np.float32)
    return [
        {"x": np.ascontiguousarray(xr[i]), "padrow": pr} for i in range(N_CORES)
    ]


def kernel(x):
    from concourse import bass_utils

    x = np.ascontiguousarray(np.asarray(x), dtype=np.float32)
    assert x.shape == (B, C, H, W)
    nc = _get_nc(IMG_PER_CORE, H, W)
    res = bass_utils.run_bass_kernel_spmd(
        nc, _in_maps(x), core_ids=list(range(N_CORES))
    )
    cnt = np.stack([res.results[i]["out"] for i in range(N_CORES)])  # [8, n_img, 64]
    return cnt.reshape(B, C, STEPS).astype(np.float32)


# revision 9
# speedup vs baseline: 1.1262x; 1.1262x over previous
"""CubECC2d Trainium kernel.

Computes per-image Euler Characteristic Curves for a cubical complex
(V-construction, sublevel filtration) over x: [128, 3, 256, 256] f32.
out[b, c, s] = #V(<=th_s) - #Eh(<=th_s) - #Ev(<=th_s) + #S(<=th_s),
i.e. the cumsum of the binned Euler characteristic histogram.

8-core data parallelism (48 images/core).  Per image the device compresses
the 4 cell arrays into a 2x2-block Euler "event" representation (6 signed
events per block instead of 16 cells: block-min, two delta-events from the
diagonal ordering, strip-mins and the corner-square max), then for each of
64 exact fp32 thresholds counts signed events in ONE fused custom-DVE
instruction (ECC_COUNT: (in0<th)-(in1<th) reduced along the free dim).
A PE ones-matmul reduces per-partition counts to the final curve.
Thresholds are precomputed on the host by bit-exact binary search so that
(v < th'_s) exactly reproduces the reference's ceil-based binning; the
device never does arithmetic on pixel values (min/max only), so the result
is bit-exact.
"""

import numpy as np

B, C, H, W = 128, 3, 256, 256
STEPS = 64
T_MIN, T_MAX = 0.02, 0.98
RES = (T_MAX - T_MIN) / (STEPS - 1)

N_CORES = 8
IMG_PER_CORE = (B * C) // N_CORES  # 48
PAD = 2.0  # sentinel > every threshold; padded slots never counted

_cache = {}


def _bin_of(v):
    """g(v) in 0..64 replicating reference binning: cell with filtration v is
    counted in cum[s] iff g(v) <= s."""
    v = np.float32(v)
    if v > np.float32(T_MAX):
        return 64
    u = (v - np.float32(T_MIN)) / np.float32(RES)
    t = int(np.ceil(u))
    return min(max(t, 0), STEPS - 1)


def _thetas_strict():
    """th'[s] = smallest float32 with g(v) > s, so (v < th'[s]) <=> g(v) <= s
    for all float32 v in [0, 1)."""
    one_bits = int(np.float32(1.0).view(np.uint32))
    out = np.empty(STEPS, np.float32)
    for s in range(STEPS):
        lo, hi = 0, one_bits - 1
        assert _bin_of(np.uint32(lo).view(np.float32)) <= s
        if _bin_of(np.uint32(hi).view(np.float32)) <= s:
            out[s] = np.float32(1.0)
            continue
        while hi - lo > 1:
            mid = (lo + hi) // 2
            if _bin_of(np.uint32(mid).view(np.float32)) <= s:
                lo = mid
            else:
                hi = mid
        out[s] = np.uint32(hi).view(np.float32)
    return out


def _build(n_img, h, w):
    """Per-core program.  Inputs: x [n_img, h, w] f32, padrow [1, w] f32.
    Output: out [n_img, 64] f32 (the finished ECC curves)."""
    from contextlib import ExitStack

    import concourse.tile as tile
    from concourse import bacc, mybir
    from ecc_op import ECC_COUNT

    assert h % 2 == 0 and w % 2 == 0
    bh, bw = h // 2, w // 2  # block grid (partitions = bh)
    fe = 3 * bw  # event-array free width

    nc = bacc.Bacc("TRN2", target_bir_lowering=False, debug=False)
    x = nc.dram_tensor("x", [n_img, h, w], mybir.dt.float32, kind="ExternalInput").ap()
    padrow = nc.dram_tensor("padrow", [1, w], mybir.dt.float32, kind="ExternalInput").ap()
    out = nc.dram_tensor("out", [n_img, 64], mybir.dt.float32, kind="ExternalOutput").ap()

    f32 = mybir.dt.float32
    OP = mybir.AluOpType
    ths = _thetas_strict()

    with tile.TileContext(nc) as tc:
        with ExitStack() as ctx:
            const_pool = ctx.enter_context(tc.tile_pool(name="const", bufs=1))
            img_pool = ctx.enter_context(tc.tile_pool(name="img", bufs=3))
            ev_pool = ctx.enter_context(tc.tile_pool(name="ev", bufs=3))
            scr_pool = ctx.enter_context(tc.tile_pool(name="scr", bufs=2))
            acc_pool = ctx.enter_context(tc.tile_pool(name="acc", bufs=3))
            psum_pool = ctx.enter_context(
                tc.tile_pool(name="psum", bufs=3, space="PSUM")
            )

            ones = const_pool.tile([bh, 1], f32)
            nc.vector.memset(ones[:], 1.0)

            for img in range(n_img):
                # X2[p, :] = [row 2p | row 2p+1]
                X2 = img_pool.tile([bh, 2 * w], f32, tag="X2")
                nc.sync.dma_start(
                    X2[:], x[img].rearrange("(i two) j -> i (two j)", two=2)
                )
                # Xe2[p, :] = row 2p+2 (p < bh-1), BIG pad at p = bh-1
                Xe2 = img_pool.tile([bh, w], f32, tag="Xe2")
                xsh = x[img].rearrange("(i two) j -> i two j", two=2)
                nc.sync.dma_start(Xe2[0 : bh - 1, :], xsh[1:bh, 0, :])
                nc.sync.dma_start(Xe2[bh - 1 : bh, :], padrow[:, :])

                Xv = X2[:].rearrange("p (two jj cc) -> p two jj cc", two=2, cc=2)
                pv = Xv[:, 0, :, 0]  # x[2i, 2j]
                qv = Xv[:, 0, :, 1]  # x[2i, 2j+1]
                rv = Xv[:, 1, :, 0]  # x[2i+1, 2j]
                sv = Xv[:, 1, :, 1]  # x[2i+1, 2j+1]
                Ev2 = Xe2[:].rearrange("p (jj cc) -> p jj cc", cc=2)
                p_b = Ev2[:, :, 0]  # x[2i+2, 2j]
                q_b = Ev2[:, :, 1]  # x[2i+2, 2j+1]

                POS = ev_pool.tile([bh, fe], f32, tag="POS")
                NEG = ev_pool.tile([bh, fe], f32, tag="NEG")
                nc.vector.memset(POS[:], PAD)
                nc.vector.memset(NEG[:], PAD)

                a1 = scr_pool.tile([bh, bw], f32, tag="a1")
                a2 = scr_pool.tile([bh, bw], f32, tag="a2")
                b1 = scr_pool.tile([bh, bw], f32, tag="b1")
                b2 = scr_pool.tile([bh, bw], f32, tag="b2")
                nc.vector.tensor_tensor(a1[:], pv, sv, op=OP.min)
                nc.vector.tensor_tensor(a2[:], pv, sv, op=OP.max)
                nc.vector.tensor_tensor(b1[:], qv, rv, op=OP.min)
                nc.vector.tensor_tensor(b2[:], qv, rv, op=OP.max)
                # y1 = global block min  (always a +1 event)
                nc.vector.tensor_tensor(POS[:, 0:bw], a1[:], b1[:], op=OP.min)
                # delta events: two smallest on one diagonal
                c1 = scr_pool.tile([bh, bw], mybir.dt.uint8, tag="c1")
                c2 = scr_pool.tile([bh, bw], mybir.dt.uint8, tag="c2")
                nc.vector.tensor_tensor(c1[:], a2[:], b1[:], op=OP.is_lt)
                nc.vector.tensor_tensor(c2[:], b2[:], a1[:], op=OP.is_lt)
                E2 = POS[:, bw : 2 * bw]
                E3 = NEG[:, 2 * bw : 3 * bw]
                nc.vector.copy_predicated(E2, c2[:], b2[:])
                nc.vector.copy_predicated(E2, c1[:], a2[:])
                nc.vector.copy_predicated(E3, c2[:], a1[:])
                nc.vector.copy_predicated(E3, c1[:], b1[:])
                # H-strip: -1 at min(max(q, p'), max(s, r')), j < bw-1
                z1 = scr_pool.tile([bh, bw], f32, tag="z1")
                z2 = scr_pool.tile([bh, bw], f32, tag="z2")
                wj = bw - 1
                nc.vector.tensor_tensor(
                    z1[:, 0:wj], Xv[:, 0, 0:wj, 1], Xv[:, 0, 1:bw, 0], op=OP.max
                )
                nc.vector.tensor_tensor(
                    z2[:, 0:wj], Xv[:, 1, 0:wj, 1], Xv[:, 1, 1:bw, 0], op=OP.max
                )
                nc.vector.tensor_tensor(
                    NEG[:, 0:wj], z1[:, 0:wj], z2[:, 0:wj], op=OP.min
                )
                # V-strip: -1 at min(max(r, p_b), max(s, q_b)), i < bh-1
                z3 = scr_pool.tile([bh, bw], f32, tag="z3")
                z4 = scr_pool.tile([bh, bw], f32, tag="z4")
                hi = bh - 1
                nc.vector.tensor_tensor(
                    z3[0:hi, :], Xv[0:hi, 1, :, 0], p_b[0:hi], op=OP.max
                )
                nc.vector.tensor_tensor(
                    z4[0:hi, :], Xv[0:hi, 1, :, 1], q_b[0:hi], op=OP.max
                )
                nc.vector.tensor_tensor(
                    NEG[0:hi, bw : bw + bw], z3[0:hi, :], z4[0:hi, :], op=OP.min
                )
                # corner: +1 at max(s, r', q_b, p_next_diag), i < bh-1, j < bw-1
                t2 = scr_pool.tile([bh, bw], f32, tag="t2")
                nc.vector.tensor_tensor(
                    t2[0:hi, 0:wj], q_b[0:hi, 0:wj], Ev2[0:hi, 1:bw, 0], op=OP.max
                )
                nc.vector.tensor_tensor(
                    POS[0:hi, 2 * bw : 2 * bw + wj],
                    z2[0:hi, 0:wj],
                    t2[0:hi, 0:wj],
                    op=OP.max,
                )

                # ---- 64-threshold sweep: one fused op per threshold
                acc = acc_pool.tile([bh, STEPS], f32, tag="acc")
                for s in range(STEPS):
                    scr = scr_pool.tile([bh, fe], f32, tag="sw")
                    nc.vector._custom_dve(
                        ECC_COUNT,
                        out=scr[:],
                        in0=POS[:],
                        in1=NEG[:],
                        s0=float(ths[s]),
                        s1=0.0,
                        accum_out=acc[:, s : s + 1],
                    )
                # partition-reduce: [1, 64] = ones.T @ acc
                ps = psum_pool.tile([1, STEPS], f32, tag="ps")
                nc.tensor.matmul(ps[:], ones[:], acc[:], start=True, stop=True)
                cur = acc_pool.tile([1, STEPS], f32, tag="cur")
                nc.scalar.copy(cur[:], ps[:])
                nc.sync.dma_start(out[img : img + 1, :], cur[:])

    nc.compile()
    return nc


def _get_nc(n_img, h, w):
    key = (n_img, h, w)
    if key not in _cache:
        _cache[key] = _build(n_img, h, w)
    return _cache[key]


def _in_maps(x):
    xr = x.reshape(N_CORES, IMG_PER_CORE, H, W)
    pr = np.full((1, W), PAD, np.float32)
    return [
        {"x": np.ascontiguousarray(xr[i]), "padrow": pr} for i in range(N_CORES)
    ]


def kernel(x):
    from concourse import bass_utils

    x = np.ascontiguousarray(np.asarray(x), dtype=np.float32)
    assert x.shape == (B, C, H, W)
    nc = _get_nc(IMG_PER_CORE, H, W)
    res = bass_utils.run_bass_kernel_spmd(
        nc, _in_maps(x), core_ids=list(range(N_CORES))
    )
    cnt = np.stack([res.results[i]["out"] for i in range(N_CORES)])  # [8, n_img, 64]
    return cnt.reshape(B, C, STEPS).astype(np.float32)
